# revision 8
# baseline (speedup 1.0000x reference)
"""Trainium2 Bass kernel for nn_CrossAttention_45286135169187 (v2).

Math (per batch b, with storage [DIM, HW], tq = w_q*target + b_q [HW]):
    u[c]      = sum_x storage[c,x] * tq[x]
    s         = sum_x tq[x]
    scores[k] = sum_c w_ca[DIM+k, c] * u[c] + b_ca[DIM+k] * s
    attn      = softmax(scores)
    vT[c]     = sum_k attn[k] * w_ca[k, c]
    beta      = sum_k attn[k] * b_ca[k]
    out[d, x] = sum_c vT[c] * storage[c,x] + beta     (identical for all d)

v2 changes vs v1:
  - storage is loaded ONCE as fp16 via SWDGE cast-DMA (no f32 tiles, no
    ACT recast pass); pass 1 runs on fp16 and pass 2 feeds the PE fp16
    directly. fp16 (not bf16) keeps the ~200-magnitude softmax logits
    accurate enough (top-2 gaps can be <1).
  - tq partition-broadcast via a K=1 PE matmul into PSUM (no DRAM
    round trip; saves 4 MB/iter of descriptor-inefficient HBM reads).
  - beta partition-broadcast likewise rides the attnT PSUM tile.
  - the output tile is kept fp16 and stored via SWDGE cast-DMA
    (fp16->f32): halves the SBUF-port read traffic of the 4x-duplicated
    channel stores; output values are fp16-rounded, well within tolerance.
  - ring assignment: gpsimd/SWDGE = cast loads + cast stores, scalar/ACT =
    small loads + PSUM->SBUF copies.
  - wv replicated as fp16; the scores path (u, wkT, s, bk) stays f32
    because logit errors are amplified by the sharp softmax.
Sharding: data-parallel over batch, 2 batches per core across 8 cores.
"""

import time

import numpy as np

import concourse.mybir as mybir
import concourse.tile as tile
from concourse import bacc, bass2jax
from concourse.bass import ts

N_CORES = 8
B = 16
DIM = 512
H = 64
W = 64
HW = H * W          # 4096
NB = B // N_CORES   # batches per core = 2
P = 128             # partitions
NCH = DIM // P      # c-chunks = 4
BLK = 512           # x-block (psum bank) size
NBLK = HW // BLK    # 8
F32 = mybir.dt.float32
F16 = mybir.dt.float16
AX_X = mybir.AxisListType.X
OP_MUL = mybir.AluOpType.mult
OP_ADD = mybir.AluOpType.add
ACT_EXP = mybir.ActivationFunctionType.Exp
ACT_COPY = mybir.ActivationFunctionType.Copy


def _emit(ctx, tc, ins, out, dbg=None, n_iters=1):
    nc = tc.nc
    storage, target, wkT, wv, bk, bv, wq, bq = ins

    def dump(name, ap):
        if dbg is not None and name in dbg:
            nc.sync.dma_start(out=dbg[name], in_=ap)

    singles = ctx.enter_context(tc.tile_pool(name="singles", bufs=1))
    stb_pool = ctx.enter_context(tc.tile_pool(name="stb", bufs=10))
    tqb_pool = ctx.enter_context(tc.tile_pool(name="tqb", bufs=2))
    outt_pool = ctx.enter_context(tc.tile_pool(name="outt", bufs=2))
    trow_pool = ctx.enter_context(tc.tile_pool(name="trow", bufs=1))
    tqbf_pool = ctx.enter_context(tc.tile_pool(name="tqbf", bufs=1))
    small_pool = ctx.enter_context(tc.tile_pool(name="small", bufs=2))
    ps_tqb = ctx.enter_context(tc.tile_pool(name="ps_tqb", bufs=2, space="PSUM"))
    ps_sc = ctx.enter_context(tc.tile_pool(name="ps_sc", bufs=1, space="PSUM"))
    ps_at = ctx.enter_context(tc.tile_pool(name="ps_at", bufs=1, space="PSUM"))
    ps_vt = ctx.enter_context(tc.tile_pool(name="ps_vt", bufs=1, space="PSUM"))
    ps_vb = ctx.enter_context(tc.tile_pool(name="ps_vb", bufs=1, space="PSUM"))
    ps_out = ctx.enter_context(tc.tile_pool(name="ps_out", bufs=2, space="PSUM"))

    # ---- replicated constants (ACT ring; loaded once, live all iters) ----
    wq_sb = singles.tile([1, 1], F32)
    nc.scalar.dma_start(out=wq_sb, in_=wq)
    bq_sb = singles.tile([1, 1], F32)
    nc.scalar.dma_start(out=bq_sb, in_=bq)
    bk_sb = singles.tile([1, DIM], F32)
    nc.scalar.dma_start(out=bk_sb, in_=bk)
    bv_sb = singles.tile([1, DIM], F32)
    nc.scalar.dma_start(out=bv_sb, in_=bv)
    wv_sb = singles.tile([P, NCH, DIM], F16)    # [p, k-chunk, c]
    wkT_sb = singles.tile([P, NCH, DIM], F32)   # [p, c-chunk, k]

    one_11 = singles.tile([1, 1], F32)          # rhs for row->column transposes
    nc.vector.memset(one_11, 1.0)
    ones_bf = singles.tile([1, P], F16)        # lhsT for K=1 partition bcast
    nc.vector.memset(ones_bf, 1.0)
    ones_f = singles.tile([1, P], F32)          # lhsT for beta bcast
    nc.vector.memset(ones_f, 1.0)
    scratch = singles.tile([P, HW], F16)       # STT mandatory full-size sink

    for it in range(n_iters):
        dbg_it = dbg if it == 0 else None

        # ---- per-batch input loads ----
        trows, stbs = [], []
        for b in range(NB):
            trow = trow_pool.tile([1, HW], F32, tag="trow")
            nc.scalar.dma_start(out=trow, in_=target[b : b + 1, :])
            trows.append(trow)
        for b in range(NB):
            st = []
            for j in range(NCH):
                t = stb_pool.tile([P, HW], F16, tag="stb")
                nc.gpsimd.dma_start(out=t, in_=storage[b, ts(j, P), :])
                st.append(t)
            stbs.append(st)

        if it == 0:
            # big weight loads after the first-pass small loads on the ACT
            # ring; they only happen once
            nc.scalar.dma_start(
                out=wv_sb, in_=wv.rearrange("(i p) c -> p i c", p=P)
            )
            nc.scalar.dma_start(
                out=wkT_sb, in_=wkT.rearrange("(j p) k -> p j k", p=P)
            )

        # ---- tq prep + partition broadcast via K=1 PE matmul ----
        tqbs, s_bfs = [], []
        for b in range(NB):
            # tq_bf <- w_q*target + b_q (bf16); s = sum(tq_bf) via a second
            # 2x-mode tensor_scalar pass with accumulate.
            trow = trows[b]
            tq_bf = tqbf_pool.tile([1, HW], F16, tag="tqbf")
            nc.vector.tensor_scalar(
                out=tq_bf, in0=trow, scalar1=wq_sb, scalar2=bq_sb,
                op0=OP_MUL, op1=OP_ADD,
            )
            s_t = small_pool.tile([1, 1], F32, tag="s")
            nc.vector.tensor_scalar(
                out=scratch[0:1, :], in0=tq_bf, scalar1=1.0, scalar2=None,
                op0=OP_MUL, op1=OP_ADD, accum_out=s_t,
            )
            s_bfs.append(s_t)
            tqb = tqb_pool.tile([P, HW], F16, tag="tqb")
            for blk in range(NBLK):
                pst = ps_tqb.tile([P, BLK], F32, tag="tqb")
                nc.tensor.matmul(
                    pst, lhsT=ones_bf, rhs=tq_bf[:, ts(blk, BLK)],
                    start=True, stop=True,
                )
                nc.scalar.copy(out=tqb[:, ts(blk, BLK)], in_=pst)
            tqbs.append(tqb)
            if b == 0 and dbg_it:
                dump("dbg_tqbf", tq_bf)
                dump("dbg_s", s_t)
                dump("dbg_tqb", tqb)

        for b in range(NB):
            st, tqb, s_bf = stbs[b], tqbs[b], s_bfs[b]

            # ---- pass 1: u[c] = <storage[c,:], tq> (bf16 DVE, f32 accum) ----
            u_t = small_pool.tile([P, NCH], F32, tag="u")
            for j in range(NCH):
                nc.vector.scalar_tensor_tensor(
                    out=scratch, in0=tqb, scalar=1.0, in1=st[j],
                    op0=OP_MUL, op1=OP_MUL, accum_out=u_t[:, j : j + 1],
                )
            if b == 0 and dbg_it:
                dump("dbg_u", u_t)

            # ---- scores row [1, DIM] = u @ wkT + s*bk (PE, accumulated) ----
            pssc = ps_sc.tile([1, DIM], F32, tag="scores")
            for j in range(NCH):
                nc.tensor.matmul(
                    pssc, lhsT=u_t[:, j : j + 1], rhs=wkT_sb[:, j, :],
                    start=(j == 0), stop=False,
                )
            nc.tensor.matmul(pssc, lhsT=s_bf, rhs=bk_sb, start=False, stop=True)

            # ---- softmax on one partition ----
            negmax = small_pool.tile([1, 1], F32, tag="negmax")
            nc.vector.reduce_max(out=negmax, in_=pssc, axis=AX_X, negate=True)
            attn = small_pool.tile([1, DIM], F32, tag="attn")
            sumexp = small_pool.tile([1, 1], F32, tag="sumexp")
            nc.scalar.activation(
                out=attn, in_=pssc, func=ACT_EXP, bias=negmax, scale=1.0,
                accum_out=sumexp,
            )
            rsum = small_pool.tile([1, 1], F32, tag="rsum")
            nc.vector.reciprocal(out=rsum, in_=sumexp)
            nc.scalar.activation(out=attn, in_=attn, func=ACT_COPY, scale=rsum)
            if b == 0 and dbg_it:
                dump("dbg_attn", attn)

            # beta = <attn, bv> (DVE accum)
            beta = small_pool.tile([1, 1], F32, tag="beta")
            nc.vector.scalar_tensor_tensor(
                out=scratch[0:1, 0:DIM], in0=attn, scalar=1.0, in1=bv_sb,
                op0=OP_MUL, op1=OP_MUL, accum_out=beta,
            )

            # ---- attn row -> columns + beta bcast, one PSUM tile ----
            psat = ps_at.tile([P, NCH + 1], F32, tag="attnT")
            for j in range(NCH):
                nc.tensor.matmul(
                    psat[:, j : j + 1], lhsT=attn[:, ts(j, P)], rhs=one_11,
                    start=True, stop=True,
                )
            nc.tensor.matmul(
                psat[:, NCH : NCH + 1], lhsT=ones_f, rhs=beta,
                start=True, stop=True,
            )
            attnT = small_pool.tile([P, NCH], F16, tag="attnTs")
            nc.vector.tensor_copy(out=attnT, in_=psat[:, 0:NCH])
            beta_col = small_pool.tile([P, 1], F32, tag="betac")
            nc.scalar.copy(out=beta_col, in_=psat[:, NCH : NCH + 1])
            if b == 0 and dbg_it:
                dump("dbg_attnT", attnT)
                dump("dbg_betac", beta_col)

            # ---- vT row [1, DIM] = attn @ wv (PE, accumulated) ----
            psvt = ps_vt.tile([1, DIM], F32, tag="vT")
            for i in range(NCH):
                nc.tensor.matmul(
                    psvt, lhsT=attnT[:, i : i + 1], rhs=wv_sb[:, i, :],
                    start=(i == 0), stop=(i == NCH - 1),
                )
            vrow = small_pool.tile([1, DIM], F16, tag="vrow")
            nc.vector.tensor_copy(out=vrow, in_=psvt)
            if b == 0 and dbg_it:
                dump("dbg_vrow", vrow)

            # vT chunks broadcast across 128 stationary columns (K=1 PE)
            vbc = small_pool.tile([P, NCH, P], F16, tag="vbc")
            for j in range(NCH):
                psvb = ps_vb.tile([P, P], F32, tag="vbc")
                nc.tensor.matmul(
                    psvb, lhsT=vrow[:, ts(j, P)], rhs=ones_bf,
                    start=True, stop=True,
                )
                nc.scalar.copy(out=vbc[:, j, :], in_=psvb)

            # ---- pass 2 (bf16): psum[d, x] = sum_c vT[c]*storage[c,x];
            # the DVE copy-out fuses the +beta ----
            ot = outt_pool.tile([P, HW], F16, tag="ot")
            for blk in range(NBLK):
                pso = ps_out.tile([P, BLK], F32, tag="pso")
                for j in range(NCH):
                    nc.tensor.matmul(
                        pso, lhsT=vbc[:, j, :], rhs=st[j][:, ts(blk, BLK)],
                        start=(j == 0), stop=(j == NCH - 1),
                    )
                nc.scalar.activation(
                    out=ot[:, ts(blk, BLK)], in_=pso,
                    func=mybir.ActivationFunctionType.Identity, bias=beta_col,
                )

            # the 512 output channels are identical -> write the same tile 4x.
            # Full-row stores: fewer SWDGE emissions beat earlier store
            # starts (HW-measured: 16 half-row stores cost ~12 us/iter more)
            for dj in range(NCH):
                nc.gpsimd.dma_start(out=out[b, ts(dj, P), :], in_=ot)


DBG_SPECS = [
    ("dbg_tqbf", [1, HW], F16), ("dbg_s", [1, 1], F32),
    ("dbg_tqb", [P, HW], F16), ("dbg_u", [P, NCH], F32),
    ("dbg_attn", [1, DIM], F32), ("dbg_attnT", [P, NCH], F16),
    ("dbg_betac", [P, 1], F32), ("dbg_vrow", [1, DIM], F16),
]


def _build_program(debug=False, n_iters=1):
    nc = bacc.Bacc(
        "TRN2", target_bir_lowering=False, debug=False, num_devices=N_CORES
    )
    storage = nc.dram_tensor("storage", [NB, DIM, HW], F32, kind="ExternalInput")
    target = nc.dram_tensor("target", [NB, HW], F32, kind="ExternalInput")
    wkT = nc.dram_tensor("wkT", [DIM, DIM], F32, kind="ExternalInput")
    wv = nc.dram_tensor("wv", [DIM, DIM], F16, kind="ExternalInput")
    bk = nc.dram_tensor("bk", [1, DIM], F32, kind="ExternalInput")
    bv = nc.dram_tensor("bv", [1, DIM], F32, kind="ExternalInput")
    wq = nc.dram_tensor("wq", [1, 1], F32, kind="ExternalInput")
    bq = nc.dram_tensor("bq", [1, 1], F32, kind="ExternalInput")
    out = nc.dram_tensor("out", [NB, DIM, HW], F32, kind="ExternalOutput")
    dbg = None
    if debug:
        dbg = {
            n: nc.dram_tensor(n, s, dt, kind="ExternalOutput").ap()
            for n, s, dt in DBG_SPECS
        }

    from contextlib import ExitStack

    with tile.TileContext(nc) as tc, ExitStack() as ctx:
        _emit(
            ctx,
            tc,
            (
                storage.ap(), target.ap(), wkT.ap(), wv.ap(),
                bk.ap(), bv.ap(), wq.ap(), bq.ap(),
            ),
            out.ap(),
            dbg=dbg,
            n_iters=n_iters,
        )
    nc.compile()
    return nc


class _Runner:
    """Jit-once PJRT executor for the compiled Bacc program (8-core SPMD)."""

    def __init__(self, nc):
        import jax
        from jax.experimental.shard_map import shard_map
        from jax.sharding import Mesh, PartitionSpec

        bass2jax.install_neuronx_cc_hook()
        self.jax = jax
        self.nc = nc
        partition_name = (
            nc.partition_id_tensor.name if nc.partition_id_tensor else None
        )
        in_names, out_names, out_avals, zero_outs = [], [], [], []
        for alloc in nc.m.functions[0].allocations:
            if not isinstance(alloc, mybir.MemoryLocationSet):
                continue
            name = alloc.memorylocations[0].name
            if alloc.kind == "ExternalInput":
                if name != partition_name:
                    in_names.append(name)
            elif alloc.kind == "ExternalOutput":
                shape = tuple(alloc.tensor_shape)
                dtype = mybir.dt.np(alloc.dtype)
                out_names.append(name)
                out_avals.append(jax.core.ShapedArray(shape, dtype))
                zero_outs.append(np.zeros(shape, dtype))
        self.in_names, self.out_names = in_names, out_names
        self.n_params = len(in_names)
        n_outs = len(out_avals)

        def _exec(params, out_bufs):
            ops = list(params) + list(out_bufs)
            if partition_name is not None:
                ops.append(bass2jax.partition_id_tensor())
            all_names = tuple(in_names) + tuple(out_names) + (
                (partition_name,) if partition_name else ()
            )
            return bass2jax._bass_exec_p.bind(
                *ops,
                out_avals=tuple(out_avals),
                in_names=all_names,
                out_names=tuple(out_names),
                lowering_input_output_aliases=(),
                sim_require_finite=True,
                sim_require_nnan=True,
                nc=nc,
            )

        def _body(*args):
            return tuple(_exec(args[: self.n_params], args[self.n_params :]))

        devices = jax.devices()[:N_CORES]
        self.mesh = Mesh(np.asarray(devices), ("core",))
        in_specs = (PartitionSpec("core"),) * (self.n_params + n_outs)
        out_specs = (PartitionSpec("core"),) * n_outs
        self.fn = jax.jit(
            shard_map(
                _body, mesh=self.mesh, in_specs=in_specs,
                out_specs=out_specs, check_rep=False,
            ),
            keep_unused=True,
        )
        self.zero_outs = zero_outs
        self._spec = PartitionSpec("core")

    def put_inputs(self, in_maps):
        import jax

        per_core = [[np.asarray(m[n]) for n in self.in_names] for m in in_maps]
        args = [
            np.concatenate([per_core[c][i] for c in range(N_CORES)], axis=0)
            for i in range(self.n_params)
        ]
        args += [np.concatenate([z] * N_CORES, axis=0) for z in self.zero_outs]
        sharding = jax.sharding.NamedSharding(self.mesh, self._spec)
        return [jax.device_put(a, sharding) for a in args]

    def run(self, dev_args):
        outs = self.fn(*dev_args)
        self.jax.block_until_ready(outs)
        return outs

    def results(self, outs):
        res = []
        for c in range(N_CORES):
            d = {}
            for i, name in enumerate(self.out_names):
                arr = np.asarray(outs[i])
                per = arr.shape[0] // N_CORES
                d[name] = arr[c * per : (c + 1) * per]
            res.append(d)
        return res


_CACHE = {}


def _get_runner(n_iters=1):
    key = n_iters
    if key not in _CACHE:
        _CACHE[key] = _Runner(_build_program(n_iters=n_iters))
    return _CACHE[key]


def _make_in_maps(storage, target, w_ca, b_ca, w_q, b_q):
    f16 = np.float16
    storage = np.asarray(storage, dtype=np.float32)
    target = np.asarray(target, dtype=np.float32)
    w_ca = np.asarray(w_ca, dtype=np.float32)
    b_ca = np.asarray(b_ca, dtype=np.float32)
    w_q = np.asarray(w_q, dtype=np.float32)
    b_q = np.asarray(b_q, dtype=np.float32)

    # host-side weight prep (tiny): split conv weight into V/K halves,
    # transpose the K half so the contraction dim lands on partitions
    wv = np.ascontiguousarray(w_ca[:DIM]).astype(f16)         # [k, c]
    wkT = np.ascontiguousarray(w_ca[DIM:].T)                  # [c, k]
    bv = b_ca[:DIM].reshape(1, DIM)
    bk = b_ca[DIM:].reshape(1, DIM)
    wq = w_q.reshape(1, 1)
    bq = b_q.reshape(1, 1)

    st_flat = storage.reshape(B, DIM, HW)
    tg_flat = target.reshape(B, HW)
    in_maps = []
    for c in range(N_CORES):
        in_maps.append(
            {
                "storage": st_flat[c * NB : (c + 1) * NB],
                "target": tg_flat[c * NB : (c + 1) * NB],
                "wkT": wkT,
                "wv": wv,
                "bk": bk,
                "bv": bv,
                "wq": wq,
                "bq": bq,
            }
        )
    return in_maps


def kernel(storage, target, w_ca, b_ca, w_q, b_q):
    runner = _get_runner()
    in_maps = _make_in_maps(storage, target, w_ca, b_ca, w_q, b_q)
    dev_args = runner.put_inputs(in_maps)
    outs = runner.run(dev_args)
    res = runner.results(outs)
    full = np.concatenate([r["out"] for r in res], axis=0)  # [B, DIM, HW]
    return full.reshape(B, DIM, H, W).astype(np.float32)


def time_kernel(storage, target, w_ca, b_ca, w_q, b_q, n_iters=33, reps=16):
    """Estimate per-execution HW time: the NEFF contains the kernel body
    unrolled n_iters times; slope vs the 1-iteration NEFF cancels the
    per-call dispatch overhead. The dispatch overhead through the PJRT
    tunnel is large (~70 ms) and jittery, so the two NEFFs are run
    alternately and the floor (min) of each distribution is used."""
    in_maps = _make_in_maps(storage, target, w_ca, b_ca, w_q, b_q)

    r1, rn = _get_runner(1), _get_runner(n_iters)
    d1, dn = r1.put_inputs(in_maps), rn.put_inputs(in_maps)
    r1.run(d1)
    rn.run(dn)
    slopes, t1_all, tn_all = [], [], []
    for round_ in range(5):
        t1s, tns = [], []
        for _ in range(reps):
            t0 = time.perf_counter()
            r1.run(d1)
            t1s.append(time.perf_counter() - t0)
            t0 = time.perf_counter()
            rn.run(dn)
            tns.append(time.perf_counter() - t0)
        t1_all += t1s
        tn_all += tns
        s = (min(tns) - min(t1s)) / (n_iters - 1)
        if s > 0:
            slopes.append(s)
        if len(slopes) >= 3:
            break
        time.sleep(1.0)
    if slopes:
        slopes.sort()
        per_exec = slopes[len(slopes) // 2]
    else:
        per_exec = max(0.0, (min(tn_all) - min(t1_all)) / (n_iters - 1))
    return per_exec, min(t1_all), min(tn_all)
